# revision 4
# baseline (speedup 1.0000x reference)
"""Trainium2 Bass kernel for nn_DynamicsBase: multi-type one-hot scatter.

Computes out[f, a, 16*t + actions[f, t, a]] = 1.0 over a zero base of shape
[2048, 256, 128] f32. Frames are sharded across 8 NeuronCores (pure data
parallelism). On each core the one-hot rows are produced by a DVE
tensor_tensor is_equal against an iota constant (broadcast access patterns,
no materialized broadcast), and streamed to HBM with ~1 MiB DMA stores.

Self-contained: hardcodes shapes; takes full inputs, returns full output.
"""
import numpy as np
from contextlib import ExitStack

import concourse.bass as bass
import concourse.tile as tile
import concourse.mybir as mybir
from concourse.bass_utils import run_bass_kernel_spmd

NUM_FRAMES, NUM_TYPES, NUM_ACTIONS = 2048, 8, 256
J = 16                      # sub-actions per type
TOTAL = NUM_TYPES * J       # 128 one-hot width
N_CORES = 8
F_PER_CORE = NUM_FRAMES // N_CORES  # 256

_CACHE = {}


def _build_nc(FB=16, out_bufs=4):
    nc = bass.Bass()
    act = nc.declare_dram_parameter(
        "actions_t", [2, 128, F_PER_CORE, NUM_TYPES], mybir.dt.float32,
        isOutput=False)
    out = nc.declare_dram_parameter(
        "out", [F_PER_CORE, NUM_ACTIONS, TOTAL], mybir.dt.float32,
        isOutput=True)

    cmod_np = np.tile(np.arange(J, dtype=np.float32)[None, :], (128, 1))
    cmod_dram = nc.inline_tensor(cmod_np, name="cmod")

    with ExitStack() as ctx:
        tc = ctx.enter_context(tile.TileContext(nc))
        const_pool = ctx.enter_context(tc.tile_pool(name="const", bufs=1))
        act_pool = ctx.enter_context(tc.tile_pool(name="act", bufs=1))
        out_pool = ctx.enter_context(tc.tile_pool(name="out", bufs=out_bufs))

        cmod_sb = const_pool.tile([128, J], mybir.dt.float32, name="cmod_sb")
        nc.sync.dma_start(cmod_sb[:], cmod_dram[:])

        act_sb = [
            act_pool.tile([128, F_PER_CORE * NUM_TYPES], mybir.dt.float32,
                          name=f"act_sb{h}", tag=f"act{h}")
            for h in range(2)
        ]
        for h in range(2):
            nc.sync.dma_start(act_sb[h][:], act[h].rearrange("a f t -> a (f t)"))

        for h in range(2):
            for fb in range(0, F_PER_CORE, FB):
                o = out_pool.tile([128, FB * TOTAL], mybir.dt.float32,
                                  name=f"o_{h}_{fb}", tag="o")
                in1 = (act_sb[h][:, fb * NUM_TYPES:(fb + FB) * NUM_TYPES]
                       .unsqueeze(2).broadcast_to([128, FB * NUM_TYPES, J]))
                in0 = (cmod_sb[:, :].unsqueeze(1)
                       .broadcast_to([128, FB * NUM_TYPES, J]))
                o_ap = o[:, :].rearrange("p (ft j) -> p ft j", j=J)
                nc.vector.tensor_tensor(o_ap, in0, in1,
                                        op=mybir.AluOpType.is_equal)
                dst = out[fb:fb + FB, h * 128:(h + 1) * 128, :].transpose(
                    [1, 0, 2])
                nc.sync.dma_start(dst, o[:, :].rearrange("p (f c) -> p f c",
                                                         c=TOTAL))
    return nc


def _split_multi_waits(nc):
    """Walrus codegen in this toolchain accepts at most ONE sync-wait per
    instruction ("Too many sync wait commands"). Tile's sem assignment can
    attach 2+. Split the extras onto same-engine NoOps placed just before
    the instruction (program order on the engine preserves semantics)."""
    def fix_block(bb):
        new = []
        for inst in bb.instructions:
            if getattr(inst, "blocks", None):
                for sub in inst.blocks:
                    fix_block(sub)
            si = inst.sync_info
            if si is not None and si.on_wait and len(si.on_wait) > 1:
                waits = list(si.on_wait)
                for k, w in enumerate(waits[:-1]):
                    nop = mybir.InstNoOp(
                        name=f"{inst.name}-waitsplit{k}",
                        engine=inst.engine,
                        ins=[], outs=[],
                        sync_info=mybir.SyncInfo(on_wait=[w], on_update=[]),
                    )
                    nc.register_instruction(nop)
                    new.append(nop)
                si.on_wait = [waits[-1]]
            new.append(inst)
        bb.instructions[:] = new
    for f in nc.m.functions:
        for bb in f.blocks:
            fix_block(bb)


def _get_nc():
    if "nc" not in _CACHE:
        nc = _build_nc()
        _split_multi_waits(nc)
        _CACHE["nc"] = nc
    return _CACHE["nc"]


def _shard_actions(actions):
    """actions [2048, 8, 256] int -> 8 per-core [2, 128, 256, 8] f32 arrays
    (a-major transpose so each SBUF partition holds one `a` row)."""
    shards = []
    for i in range(N_CORES):
        sh = actions[i * F_PER_CORE:(i + 1) * F_PER_CORE]   # [256f, 8t, 256a]
        at = np.ascontiguousarray(sh.transpose(2, 0, 1))    # [256a, 256f, 8t]
        shards.append(
            at.reshape(2, 128, F_PER_CORE, NUM_TYPES).astype(np.float32))
    return shards


def kernel(actions, base, _trace=False, _trace_kwargs=None):
    actions = np.asarray(actions)
    base = np.asarray(base)
    assert actions.shape == (NUM_FRAMES, NUM_TYPES, NUM_ACTIONS), actions.shape
    nc = _get_nc()
    in_maps = [{"actions_t": s} for s in _shard_actions(actions)]
    kw = {}
    if _trace:
        kw = dict(trace=True, trace_kwargs=_trace_kwargs or {})
    res = run_bass_kernel_spmd(nc, in_maps, core_ids=list(range(N_CORES)), **kw)
    out = np.concatenate([r["out"] for r in res.results], axis=0)
    if _trace:
        kernel.last_results = res
    return out.astype(base.dtype, copy=False)
